# revision 52
# baseline (speedup 1.0000x reference)
"""Autoformer encoder block on 8 TRN2 NeuronCores.

Sharding: data-parallel over batch (B=8 -> 1 batch per core), weights
replicated. No collectives.

Per-core math (S=1024, D=512, H=8, dp=64, K=25):
  trend = movavg(x)               # banded matmul, token-major
  seas  = x - trend               # token-major, then PE-transpose -> seas.T
  q.T/k.T = wq/wk.T @ seas.T      # feature-major
  v     = seas @ wv               # token-major (for AV lhsT + V-sums)
  The reference's rfft/irfft over the depth axis (n=2S) makes
  corr[b,h,s,t] == 0 for t >= dp, so attention reduces to 64 depth-lags:
    corr.T = IDFT @ (QF (*) conj(KF)), QF = FWD.T @ q.T   (n=128 DFT)
    E = exp(corr/8); out = (E @ v[:64] + (Vsum - Vhead)) / (rowsum(E)+S-dp)
  wo, LN1, FFN(4x, relu), LN2 feature-major (stats via ones-matmul).
  seasonal_out + trend == x_out exactly (trend2 cancels), so movavg2 is
  skipped; final LN3 runs token-major after a PE-transpose, then DMA out.
"""

import numpy as np

B, S, D, H = 8, 1024, 512, 8
DP = D // H
DH = 4 * D
KWIN, PAD = 25, 12
EPS = 1e-6
NCORES = 8
NT = S // 128   # 8 token tiles
ND = D // 128   # 4 feature tiles
NH = DH // 128  # 16 hidden tiles

_CACHE = {}


def _consts():
    c = {}
    # moving-average band blocks: trend[s,:] = sum_t A[t,s] x[t,:]
    # piece 1: t = 128j-12+i, i in [0,128)  (rhs = x_m12 block j)
    # piece 2: t = 128j+116+i, i in [0,24)  (rhs = x_m12 block j+1, rows 0:24)
    cnt = np.minimum(S, np.arange(S) + PAD + 1) - np.maximum(0, np.arange(S) - PAD)
    BAND1 = np.zeros((128, NT * 128), np.float32)
    BAND2 = np.zeros((2 * PAD, NT * 128), np.float32)
    for j in range(NT):
        for cc in range(128):
            s = 128 * j + cc
            for i in range(128):
                t = 128 * j - PAD + i
                if 0 <= t < S and abs(t - s) <= PAD:
                    BAND1[i, 128 * j + cc] = 1.0 / cnt[s]
            for i in range(2 * PAD):
                t = 128 * j + 116 + i
                if 0 <= t < S and abs(t - s) <= PAD:
                    BAND2[i, 128 * j + cc] = 1.0 / cnt[s]
    c["BAND1"], c["BAND2"] = BAND1, BAND2

    # split forward DFT (n=128), doubled over partitions for base-0/64 heads:
    # FWDC [128, 65]: cos(2pi f dd/128), f=0..64; FWDS [128, 64]: sin, f=0..63
    n = 2 * DP
    dd = np.arange(DP)[:, None]
    FWDC = np.cos(2 * np.pi * np.arange(65)[None, :] * dd / n).astype(np.float32)
    FWDS = np.sin(2 * np.pi * np.arange(65)[None, :] * dd / n).astype(np.float32)
    c["FWDC"] = np.concatenate([FWDC, FWDC], axis=0)
    c["FWDS"] = np.concatenate([FWDS, FWDS], axis=0)

    # inverse: corr[t] = IDFT_RE.T @ P_re(f=0..64) + IDFT_IM.T @ P_im(f=0..63)
    t = np.arange(DP)[None, :]
    w = np.full(65, 2.0); w[0] = 1.0; w[64] = 1.0
    fr = np.arange(65)[:, None]
    IRE = (w[:, None] / n) * np.cos(2 * np.pi * fr * t / n)
    fi = np.arange(65)[:, None]
    IIM = -(2.0 / n) * np.sin(2 * np.pi * fi * t / n)
    c["IRE"] = IRE.astype(np.float32)
    c["IIM"] = IIM.astype(np.float32)

    # head-pair selector for 1/Z broadcast: pass p covers heads 2p, 2p+1
    HSEL = np.zeros((8, 4 * 128), np.float32)
    for p in range(4):
        for mm_ in range(128):
            HSEL[2 * p + mm_ // 64, p * 128 + mm_] = 1.0
    c["HSEL"] = HSEL
    # Z-collect one-hot: EH8[:, 8h+m] = (m == h)
    EH8 = np.zeros((128, 64), np.float32)
    for h in range(8):
        EH8[:, 9 * h] = 1.0
    c["EH8"] = EH8

    c["ones"] = np.ones((128, 128), np.float32)
    c["ident"] = np.eye(128, dtype=np.float32)
    return c


def _build():
    import concourse.bacc as bacc
    import concourse.mybir as mybir
    import concourse.tile as tile

    f32 = mybir.dt.float32
    f32r = mybir.dt.float32r
    Alu = mybir.AluOpType
    Act = mybir.ActivationFunctionType

    nc = bacc.Bacc()

    # ---- DRAM parameters (same names as setup_inputs keys) ----
    x_d = nc.declare_dram_parameter("x", [S, D], f32, isOutput=False)
    wq_d = nc.declare_dram_parameter("wq", [D, D], f32, isOutput=False)
    bq_d = nc.declare_dram_parameter("bq", [D], f32, isOutput=False)
    wk_d = nc.declare_dram_parameter("wk", [D, D], f32, isOutput=False)
    bk_d = nc.declare_dram_parameter("bk", [D], f32, isOutput=False)
    wv_d = nc.declare_dram_parameter("wv", [D, D], f32, isOutput=False)
    bv_d = nc.declare_dram_parameter("bv", [D], f32, isOutput=False)
    wo_d = nc.declare_dram_parameter("wo", [D, D], f32, isOutput=False)
    bo_d = nc.declare_dram_parameter("bo", [D], f32, isOutput=False)
    w1_d = nc.declare_dram_parameter("w1", [D, DH], f32, isOutput=False)
    b1_d = nc.declare_dram_parameter("b1", [DH], f32, isOutput=False)
    w2_d = nc.declare_dram_parameter("w2", [DH, D], f32, isOutput=False)
    b2_d = nc.declare_dram_parameter("b2", [D], f32, isOutput=False)
    g1_d = nc.declare_dram_parameter("g1", [D], f32, isOutput=False)
    be1_d = nc.declare_dram_parameter("be1", [D], f32, isOutput=False)
    g2_d = nc.declare_dram_parameter("g2", [D], f32, isOutput=False)
    be2_d = nc.declare_dram_parameter("be2", [D], f32, isOutput=False)
    g3_d = nc.declare_dram_parameter("g3", [D], f32, isOutput=False)
    be3_d = nc.declare_dram_parameter("be3", [D], f32, isOutput=False)
    out_d = nc.declare_dram_parameter("out", [S, D], f32, isOutput=True)

    C = _consts()
    B1_d = nc.inline_tensor(C["BAND1"], name="c_band1")
    B2_d = nc.inline_tensor(C["BAND2"], name="c_band2")
    FWDC_d = nc.inline_tensor(C["FWDC"], name="c_fwdc")
    FWDS_d = nc.inline_tensor(C["FWDS"], name="c_fwds")
    IRE_d = nc.inline_tensor(C["IRE"], name="c_ire")
    IIM_d = nc.inline_tensor(C["IIM"], name="c_iim")
    HSEL_d = nc.inline_tensor(C["HSEL"], name="c_hsel")
    EH8_d = nc.inline_tensor(C["EH8"], name="c_eh8")
    ones_d = nc.inline_tensor(C["ones"], name="c_ones")
    ident_d = nc.inline_tensor(C["ident"], name="c_ident")

    def r(ap):
        return ap.bitcast(f32r)

    def mm(out, lhsT, rhs, start=True, stop=True):
        nc.tensor.matmul(out, r(lhsT), r(rhs), start=start, stop=stop)

    with tile.TileContext(nc) as tc:
        with (
            tc.tile_pool(name="konst", bufs=1) as konst,
            tc.tile_pool(name="persist", bufs=1) as persist,
            tc.tile_pool(name="mid", bufs=1) as mid,
        ):
            # ---- consts to SBUF ----
            fwdc = konst.tile([128, 65], f32)
            fwds = konst.tile([128, 65], f32)
            ire = konst.tile([65, DP], f32)
            iim = konst.tile([65, DP], f32)
            hsel = konst.tile([8, 4 * 128], f32)
            eh8 = konst.tile([128, 64], f32)
            ones = konst.tile([128, 128], f32)
            ident = konst.tile([128, 128], f32)
            for tl, dr in ((fwdc, FWDC_d), (fwds, FWDS_d), (ire, IRE_d),
                           (iim, IIM_d), (hsel, HSEL_d), (eh8, EH8_d),
                           (ones, ones_d)):
                nc.sync.dma_start(r(tl[:]), r(dr[:]))
            nc.sync.dma_start(ident[:], ident_d[:])

            def col(dram, n):
                t = konst.tile([128, n], f32, tag="col_" + dram.name)
                nc.sync.dma_start(t[:], dram.rearrange("(j p) -> p j", p=128))
                return t
            bq_c, bk_c, bv_c = col(bq_d, ND), col(bk_d, ND), col(bv_d, ND)
            bo_c, b2_c = col(bo_d, ND), col(b2_d, ND)
            b1_c = col(b1_d, NH)
            g1_c, be1_c = col(g1_d, ND), col(be1_d, ND)
            g2_c, be2_c = col(g2_d, ND), col(be2_d, ND)
            g3_r = konst.tile([1, D], f32)
            be3_r = konst.tile([1, D], f32)
            nc.sync.dma_start(r(g3_r[:]), r(g3_d[None, :]))
            nc.sync.dma_start(r(be3_r[:]), r(be3_d[None, :]))

            trend_tok = persist.tile([128, NT * D], f32)
            seasT = persist.tile([128, ND * S], f32)
            wqkvo = []

            def wload(pool, dram, din, dout):
                t = pool.tile([128, (din // 128) * dout], f32,
                              tag="w_" + dram.name)
                nc.sync.dma_start(
                    r(t[:].rearrange("p (k f) -> p k f", f=dout)),
                    r(dram.rearrange("(k p) f -> p k f", p=128)))
                return t

            # ============ phase 1: load x, moving average, transpose ========
            wop_p = tc.tile_pool(name="wop", bufs=1)
            wop = wop_p.__enter__()
            wat_ctx = tc.tile_pool(name="wat", bufs=1)
            wat = wat_ctx.__enter__()
            with (
                tc.tile_pool(name="ph1", bufs=1) as ph1,
                tc.tile_pool(name="ps1", bufs=2, space="PSUM") as ps1,
            ):
                band1 = ph1.tile([128, NT * 128], f32)
                band2 = ph1.tile([2 * PAD, NT * 128], f32)
                nc.sync.dma_start(r(band1[:]), r(B1_d[:]))
                nc.sync.dma_start(r(band2[:]), r(B2_d[:]))
                x_m12 = ph1.tile([128, (NT + 1) * D], f32)
                nc.sync.dma_start(r(x_m12[0:PAD, 0:D]), r(x_d[0:PAD, :]))
                nc.sync.dma_start(r(x_m12[PAD:128, 0:D]), r(x_d[0:116, :]))
                nc.sync.dma_start(
                    r(x_m12[:, D:4 * D].rearrange("p (st d) -> p st d", d=D)),
                    r(x_d[116:116 + 3 * 128, :].rearrange(
                        "(st p) d -> p st d", p=128)))
                nc.sync.dma_start(
                    r(x_m12[:, 4 * D:NT * D].rearrange("p (st d) -> p st d", d=D)),
                    r(x_d[116 + 3 * 128:116 + 7 * 128, :].rearrange(
                        "(st p) d -> p st d", p=128)))
                nc.sync.dma_start(r(x_m12[0:PAD, NT * D:(NT + 1) * D]),
                                  r(x_d[S - PAD:S, :]))
                nc.sync.dma_start(r(x_m12[PAD:128, NT * D:(NT + 1) * D]),
                                  r(x_d[0:116, :]))
                x_tok = ph1.tile([128, NT * D], f32)
                for st_ in range(NT):
                    nc.sync.dma_start(
                        x_tok[:, st_ * D:(st_ + 1) * D],
                        x_d[st_ * 128:(st_ + 1) * 128, :])
                for dr_ in (wq_d, wk_d, wv_d):
                    wqkvo.append(wload(wat, dr_, D, D))
                wqkvo.append(wload(wop, wo_d, D, D))
                seas_tok = ph1.tile([128, NT * D], f32)
                for j in range(NT):
                    ps = ps1.tile([128, D], f32, tag="mavg")
                    mm(ps[:], band1[:, 128 * j:128 * (j + 1)],
                       x_m12[:, j * D:(j + 1) * D], start=True, stop=False)
                    mm(ps[:], band2[:, 128 * j:128 * (j + 1)],
                       x_m12[0:2 * PAD, (j + 1) * D:(j + 2) * D],
                       start=False, stop=True)
                    nc.scalar.copy(trend_tok[:, j * D:(j + 1) * D], ps[:])
                    nc.vector.tensor_tensor(
                        seas_tok[:, j * D:(j + 1) * D],
                        x_tok[:, j * D:(j + 1) * D], ps[:], Alu.subtract)

                for st in range(NT):
                    for kt in range(ND):
                        ps = ps1.tile([128, 128], f32, tag="tr")
                        nc.tensor.transpose(
                            ps[:],
                            seas_tok[:, st * D + kt * 128:st * D + (kt + 1) * 128],
                            ident[:])
                        dst = r(seasT[:, kt * S + st * 128:kt * S + (st + 1) * 128])
                        if (st + kt) % 2 == 0:
                            nc.vector.tensor_copy(dst, ps[:])
                        else:
                            nc.scalar.copy(dst, ps[:])

            # ============ phase 2: attention ============
            out1T = mid.tile([128, ND * S], f32, tag="m16")
            with (
                tc.tile_pool(name="ph2", bufs=1) as ph2,
                tc.tile_pool(name="att1", bufs=1) as att1,
                tc.tile_pool(name="scr2", bufs=2) as scr2,
                tc.tile_pool(name="scr2a", bufs=1) as scr2a,
            ):
                wq_s, wk_s, wv_s, wo_s = wqkvo
                bf = mybir.dt.bfloat16
                qT = ph2.tile([128, ND * S], bf)
                kT = ph2.tile([128, ND * S], bf)
                v_tok = ph2.tile([128, NT * D], f32)
                attnT = mid.tile([128, ND * S], f32, tag="attn")

                with tc.tile_pool(name="psqkv", bufs=2, space="PSUM") as psq:
                    for w_s, bcol, dst in ((wq_s, bq_c, qT), (wk_s, bk_c, kT)):
                        for mt in range(ND):
                            ps = psq.tile([128, S], f32, tag="big")
                            for nn in range(2):
                                for k in range(ND):
                                    mm(ps[:, nn * 512:(nn + 1) * 512],
                                       w_s[:, k * D + mt * 128:k * D + (mt + 1) * 128],
                                       seasT[:, k * S + nn * 512:k * S + (nn + 1) * 512],
                                       start=(k == 0), stop=(k == ND - 1))
                            nc.scalar.activation(
                                dst[:, mt * S:(mt + 1) * S], ps[:],
                                Act.Identity, bias=bcol[:, mt:mt + 1], scale=1.0)
                    for st in range(NT):
                        ps = psq.tile([128, D], f32, tag="vtok")
                        for k in range(ND):
                            mm(ps[:], seasT[:, k * S + st * 128:k * S + (st + 1) * 128],
                               wv_s[:, k * D:(k + 1) * D],
                               start=(k == 0), stop=(k == ND - 1))
                        nc.scalar.copy(r(v_tok[:, st * D:(st + 1) * D]), ps[:])

                ire_b = att1.tile([65, DP], bf)
                iim_b = att1.tile([65, DP], bf)
                fwdc_b = att1.tile([128, 65], bf)
                fwds_b = att1.tile([128, 65], bf)
                nc.vector.tensor_copy(ire_b[:], ire[:])
                nc.vector.tensor_copy(iim_b[:], iim[:])
                nc.vector.tensor_copy(fwdc_b[:], fwdc[:])
                nc.vector.tensor_copy(fwds_b[:], fwds[:])
                E_all = att1.tile([DP, H * S], bf)
                scale = float(1.0 / np.sqrt(DP))

                # phase A: DFT -> complex product (bf16) -> corr -> E
                with tc.tile_pool(name="psA2", bufs=2, space="PSUM") as psA2:
                    for h in range(H):
                        kt, ro = h // 2, 64 * (h % 2)
                        odd = (h % 2 == 1)
                        fc = fwdc_b[0:DP, :] if not odd else fwdc_b[DP:128, :]
                        fs = fwds_b[0:DP, :] if not odd else fwds_b[DP:128, :]
                        c0 = kt * S
                        qh = qT[ro:ro + DP, c0:c0 + S]
                        kh = kT[ro:ro + DP, c0:c0 + S]
                        qc = psA2.tile([65, S], f32, tag="fq")
                        qsn = psA2.tile([65, S], f32, tag="fq")
                        kc = psA2.tile([65, S], f32, tag="fk")
                        ksn = psA2.tile([65, S], f32, tag="fk")
                        for nn in range(2):
                            sl = slice(nn * 512, (nn + 1) * 512)
                            nc.tensor.matmul(qc[:, sl], fc, qh[:, sl],
                                             start=True, stop=True)
                            nc.tensor.matmul(qsn[:, sl], fs, qh[:, sl],
                                             start=True, stop=True)
                            nc.tensor.matmul(kc[:, sl], fc, kh[:, sl],
                                             start=True, stop=True)
                            nc.tensor.matmul(ksn[:, sl], fs, kh[:, sl],
                                             start=True, stop=True)
                        qcb = scr2.tile([65, S], bf, tag="qcb")
                        qsb = scr2.tile([65, S], bf, tag="qsb")
                        kcb = scr2.tile([65, S], bf, tag="kcb")
                        ksb = scr2a.tile([65, S], bf, tag="ksb")
                        nc.vector.tensor_copy(qcb[:], qc[:])
                        nc.scalar.copy(qsb[:], qsn[:])
                        nc.vector.tensor_copy(kcb[:], kc[:])
                        nc.scalar.copy(ksb[:], ksn[:])
                        pre = scr2a.tile([65, S], bf, tag="pre")
                        pim = scr2a.tile([65, S], bf, tag="pim")
                        t2 = scr2a.tile([65, S], bf, tag="t2")
                        nc.vector.tensor_tensor(pre[:], qcb[:], kcb[:],
                                                Alu.mult)
                        nc.gpsimd.tensor_tensor(t2[:], qsb[:], ksb[:],
                                                Alu.mult)
                        nc.vector.tensor_tensor(pre[:], pre[:], t2[:], Alu.add)
                        nc.vector.tensor_tensor(pim[:], qcb[:], ksb[:],
                                                Alu.mult)
                        nc.vector.tensor_tensor(t2[:], qsb[:], kcb[:],
                                                Alu.mult)
                        nc.vector.tensor_tensor(pim[:], pim[:], t2[:],
                                                Alu.subtract)
                        cr = psA2.tile([DP, S], f32, tag="fk")
                        for nn in range(2):
                            sl = slice(nn * 512, (nn + 1) * 512)
                            nc.tensor.matmul(cr[:, sl], ire_b[:], pre[:, sl],
                                             start=True, stop=False)
                            nc.tensor.matmul(cr[:, sl], iim_b[:], pim[:, sl],
                                             start=False, stop=True)
                        nc.scalar.activation(E_all[:, h * S:(h + 1) * S],
                                             cr[:], Act.Exp, bias=0.0,
                                             scale=scale)

                # W = Vsum - Vhead = (sum_{s>=64} seas.T[:,s]) @ wv
                W_sb = att1.tile([128, ND], f32)
                sdif = att1.tile([128, 2 * ND], f32)
                dmp = scr2a.tile([128, S - DP], f32, tag="dmp")
                for k in range(ND):
                    with nc.allow_low_precision(reason="f32r rhs"):
                        nc.scalar.activation(
                            dmp[:], seasT[:, k * S + DP:(k + 1) * S],
                            Act.Copy,
                            accum_out=r(sdif[:, 2 * k:2 * k + 1]))
                    nc.vector.tensor_copy(r(sdif[:, 2 * k + 1:2 * k + 2]),
                                          sdif[:, 2 * k:2 * k + 1])
                with tc.tile_pool(name="psw", bufs=1, space="PSUM") as psw:
                    ps_w2 = psw.tile([128, 2 * ND], f32, tag="w2")
                    for mt in range(ND):
                        for k in range(ND):
                            mm(ps_w2[:, 2 * mt:2 * mt + 2],
                               wv_s[:, k * D + mt * 128:k * D + (mt + 1) * 128],
                               sdif[:, 2 * k:2 * k + 2],
                               start=(k == 0), stop=(k == ND - 1))
                    nc.vector.tensor_copy(W_sb[:],
                                          ps_w2[:].rearrange("p (a b) -> p a b", b=2)[:, :, 0])

                # phase B: AV + Z-collect -> batched 1/Z -> scale + bv
                vb = att1.tile([DP, H * DP + 1], bf)
                nc.vector.tensor_copy(vb[:, 0:H * DP], v_tok[0:DP, 0:H * DP])
                nc.vector.tensor_copy(vb[:, H * DP:H * DP + 1], ones[0:DP, 0:1])
                zrs = att1.tile([128, S], f32)
                with (
                    tc.tile_pool(name="psB2", bufs=2, space="PSUM") as psB2,
                    tc.tile_pool(name="psZ", bufs=1, space="PSUM") as psZ,
                ):
                    Zall = psZ.tile([8, S], f32, tag="zall")
                    for h in range(H):
                        kt, ro = h // 2, 64 * (h % 2)
                        odd = (h % 2 == 1)
                        c0 = kt * S
                        if not odd:
                            avl = scr2.tile([DP, 65], bf, tag="avle")
                            nc.vector.tensor_copy(
                                avl[:, 0:DP], vb[:, DP * h:DP * h + DP])
                            nc.vector.tensor_copy(
                                avl[:, DP:65], vb[:, H * DP:H * DP + 1])
                        else:
                            avl = scr2.tile([DP, 128], bf, tag="avlo")
                            nc.vector.tensor_copy(
                                avl[:, 0:1], vb[:, H * DP:H * DP + 1])
                            nc.vector.tensor_scalar(avl[:, 1:64],
                                                    vb[:, 1:64], 0.0,
                                                    None, Alu.mult)
                            nc.vector.tensor_copy(
                                avl[:, 64:128], vb[:, DP * h:DP * h + DP])
                        nv = psB2.tile([128, S], f32, tag="nv")
                        for nn in range(2):
                            sl = slice(nn * 512, (nn + 1) * 512)
                            nc.tensor.matmul(
                                nv[0:avl.shape[1], sl], avl[:],
                                E_all[:, h * S + nn * 512:h * S + (nn + 1) * 512],
                                start=True, stop=True)
                        zrow = 64 if not odd else 0
                        nrows = slice(0, DP) if not odd else slice(64, 128)
                        nc.scalar.activation(r(zrs[zrow:zrow + 1, :]),
                                             nv[zrow:zrow + 1, :], Act.Copy)
                        for nn in range(2):
                            sl = slice(nn * 512, (nn + 1) * 512)
                            mm(Zall[:, sl], eh8[zrow:zrow + 1, 8 * h:8 * h + 8],
                               zrs[zrow:zrow + 1, sl],
                               start=(h == 0), stop=(h == H - 1))
                        nc.vector.tensor_scalar(
                            r(attnT[ro:ro + DP, c0:c0 + S]), nv[nrows, :],
                            W_sb[ro:ro + DP, kt:kt + 1], None, Alu.add)
                    zsum = att1.tile([8, S], f32)
                    zinv = att1.tile([8, S], f32)
                    zinv_b = att1.tile([8, S], bf)
                    hsel_b = att1.tile([8, 4 * 128], bf)
                    nc.vector.tensor_copy(hsel_b[:], hsel[:])
                    nc.vector.tensor_scalar(zsum[:], Zall[:], float(S - DP),
                                            None, Alu.add)
                    nc.scalar.activation(zinv[:], zsum[:], Act.Ln, bias=0.0,
                                         scale=1.0)
                    nc.scalar.activation(zinv_b[:], zinv[:], Act.Exp, bias=0.0,
                                         scale=-1.0)
                with tc.tile_pool(name="psZb", bufs=2, space="PSUM") as psZb:
                    for p in range(4):
                        zbc = psZb.tile([128, S], f32, tag="zbc")
                        for nn in range(2):
                            sl = slice(nn * 512, (nn + 1) * 512)
                            nc.tensor.matmul(zbc[:, sl],
                                             hsel_b[:, p * 128:(p + 1) * 128],
                                             zinv_b[:, sl],
                                             start=True, stop=True)
                        nc.vector.tensor_tensor(r(attnT[:, p * S:(p + 1) * S]),
                                                attnT[:, p * S:(p + 1) * S],
                                                zbc[:], Alu.mult)
                        nc.vector.tensor_scalar(r(attnT[:, p * S:(p + 1) * S]),
                                                attnT[:, p * S:(p + 1) * S],
                                                bv_c[:, p:p + 1], None, Alu.add)

            wat_ctx.__exit__(None, None, None)
            out1T_ = out1T

            # ============ layernorm helper (feature-major, per token-half) ==
            def layernorm_T(psln, scr, scr1, src, dst, g_c, be_c, h0):
                if True:
                    s1 = psln.tile([1, 512], f32, tag="stat1")
                    s2 = psln.tile([1, 512], f32, tag="stat2")
                    for k in range(ND):
                        c0 = k * S + h0
                        sq = scr.tile([128, 512], f32, tag="lnsq")
                        nc.vector.tensor_tensor(r(sq[:]), src[:, c0:c0 + 512],
                                                src[:, c0:c0 + 512], Alu.mult)
                        mm(s1[:], ones[:, 0:1], src[:, c0:c0 + 512],
                           start=(k == 0), stop=(k == ND - 1))
                        mm(s2[:], ones[:, 0:1], sq[:],
                           start=(k == 0), stop=(k == ND - 1))
                    mean = scr1.tile([1, 512], f32, tag="lnm")
                    msq = scr1.tile([1, 512], f32, tag="lnq")
                    var = scr1.tile([1, 512], f32, tag="lnv")
                    sd = scr1.tile([1, 512], f32, tag="lnq")
                    rstd = scr1.tile([1, 512], f32, tag="lnr")
                    bbn = scr1.tile([1, 512], f32, tag="lnv")
                    bb = scr1.tile([1, 512], f32, tag="lnm")
                    nc.vector.tensor_scalar(mean[:], s1[:], 1.0 / D, None,
                                            Alu.mult)
                    nc.vector.tensor_scalar(var[:], s2[:], 1.0 / D, EPS,
                                            Alu.mult, Alu.add)
                    nc.vector.tensor_tensor(msq[:], mean[:], mean[:], Alu.mult)
                    nc.vector.tensor_tensor(var[:], var[:], msq[:],
                                            Alu.subtract)
                    nc.scalar.activation(sd[:], var[:], Act.Ln, bias=0.0,
                                         scale=1.0)
                    nc.scalar.activation(r(rstd[:]), sd[:], Act.Exp, bias=0.0,
                                         scale=-0.5)
                    nc.vector.tensor_scalar(bbn[:], mean[:], -1.0, None,
                                            Alu.mult)
                    nc.vector.tensor_tensor(r(bb[:]), bbn[:], rstd[:], Alu.mult)
                    abc = psln.tile([128, 512], f32, tag="lnA")
                    bbc = psln.tile([128, 512], f32, tag="lnB")
                    mm(abc[:], ones[0:1, :], rstd[:])
                    mm(bbc[:], ones[0:1, :], bb[:])
                    for k in range(ND):
                        c0 = k * S + h0
                        t = scr.tile([128, 512], f32, tag="lnt")
                        nc.vector.tensor_tensor(t[:], src[:, c0:c0 + 512],
                                                abc[:], Alu.mult)
                        nc.vector.tensor_tensor(t[:], t[:], bbc[:], Alu.add)
                        nc.scalar.activation(r(dst[:, c0:c0 + 512]), t[:],
                                             Act.Identity,
                                             bias=be_c[:, k:k + 1],
                                             scale=g_c[:, k:k + 1])

            # ============ back end: wo -> LN1 -> FFN -> LN2 -> LN3, 2 halves =
            with (
                tc.tile_pool(name="ph3", bufs=2) as ph3,
                tc.tile_pool(name="scr3", bufs=2) as scr3,
                tc.tile_pool(name="scr3s", bufs=1) as scr3s,
                tc.tile_pool(name="w1p", bufs=1) as w1p,
                tc.tile_pool(name="w2p", bufs=1) as w2p,
                tc.tile_pool(name="hTp", bufs=1) as hTp,
                tc.tile_pool(name="psbig", bufs=4, space="PSUM") as psbig,
                tc.tile_pool(name="psst", bufs=1, space="PSUM") as psst,
            ):
                w1_s = wload(w1p, w1_d, D, DH)
                w2_s = wload(w2p, w2_d, DH, D)
                sum2T = attnT
                g3p = psbig.tile([128, D], f32, tag="big")
                be3p = psbig.tile([128, D], f32, tag="big")
                mm(g3p[:], ones[0:1, :], g3_r[:])
                mm(be3p[:], ones[0:1, :], be3_r[:])
                g3bc = scr3s.tile([128, D], f32, tag="g3bc")
                be3bc = scr3s.tile([128, D], f32, tag="be3bc")
                nc.vector.tensor_copy(g3bc[:], g3p[:])
                nc.vector.tensor_copy(be3bc[:], be3p[:])
                stat = scr3s.tile([128, NT], f32, tag="st3s")
                statq = scr3s.tile([128, NT], f32, tag="st3q")
                mean3 = scr3s.tile([128, NT], f32, tag="st3m")
                rstd3 = scr3s.tile([128, NT], f32, tag="st3r")
                nb3 = scr3s.tile([128, NT], f32, tag="st3nb")
                dump = scr3.tile([128, D], f32, tag="xn")

                for hf in range(2):
                    h0 = hf * 512
                    # wo + residual
                    if True:
                        for mt in range(ND):
                            ps = psbig.tile([128, 512], f32, tag="big")
                            for k in range(ND):
                                mm(ps[:],
                                   wo_s[:, k * D + mt * 128:k * D + (mt + 1) * 128],
                                   attnT[:, k * S + h0:k * S + h0 + 512],
                                   start=(k == 0), stop=(k == ND - 1))
                            tmp = scr3s.tile([128, 512], f32, tag="evac")
                            nc.scalar.activation(tmp[:], ps[:], Act.Identity,
                                                 bias=bo_c[:, mt:mt + 1],
                                                 scale=1.0)
                            nc.vector.tensor_tensor(
                                r(out1T_[:, mt * S + h0:mt * S + h0 + 512]),
                                tmp[:], seasT[:, mt * S + h0:mt * S + h0 + 512],
                                Alu.add)
                    layernorm_T(psst, scr3, scr3s, out1T_, out1T_, g1_c, be1_c, h0)
                    # FFN
                    if True:
                        hTa = hTp.tile([128, 8 * 512], f32, tag="hTa")
                        hTb = hTp.tile([128, 8 * 512], f32, tag="hTb")
                        hTs = [hTa, hTb]
                        for mt in range(NH):
                            ps = psbig.tile([128, 512], f32, tag="big")
                            for k in range(ND):
                                mm(ps[:],
                                   w1_s[:, k * DH + mt * 128:k * DH + (mt + 1) * 128],
                                   out1T_[:, k * S + h0:k * S + h0 + 512],
                                   start=(k == 0), stop=(k == ND - 1))
                            ht_dst = hTs[mt // 8][:, (mt % 8) * 512:(mt % 8 + 1) * 512]
                            if mt % 2 == 0:
                                nc.scalar.activation(
                                    r(ht_dst), ps[:],
                                    Act.Relu, bias=b1_c[:, mt:mt + 1], scale=1.0)
                            else:
                                nc.vector.tensor_scalar(
                                    r(ht_dst), ps[:],
                                    b1_c[:, mt:mt + 1], 0.0, Alu.add, Alu.max)
                        for mt in range(ND):
                            ps = psbig.tile([128, 512], f32, tag="big")
                            for k in range(NH):
                                mm(ps[:],
                                   w2_s[:, k * D + mt * 128:k * D + (mt + 1) * 128],
                                   hTs[k // 8][:, (k % 8) * 512:(k % 8 + 1) * 512],
                                   start=(k == 0), stop=(k == NH - 1))
                            tmp = scr3s.tile([128, 512], f32, tag="evac")
                            nc.scalar.activation(tmp[:], ps[:], Act.Identity,
                                                 bias=b2_c[:, mt:mt + 1],
                                                 scale=1.0)
                            nc.vector.tensor_tensor(
                                r(sum2T[:, mt * S + h0:mt * S + h0 + 512]),
                                tmp[:],
                                out1T_[:, mt * S + h0:mt * S + h0 + 512],
                                Alu.add)
                    layernorm_T(psst, scr3, scr3s, sum2T, sum2T, g2_c, be2_c, h0)
                    for k in range(ND):
                        c0 = k * S + h0
                        nc.vector.tensor_tensor(r(sum2T[:, c0:c0 + 512]),
                                                sum2T[:, c0:c0 + 512],
                                                seasT[:, c0:c0 + 512], Alu.add)
                    # transpose back + trend, LN3 token-major, store
                    if True:
                        for st in range(hf * 4, hf * 4 + 4):
                            xo_st = ph3.tile([128, D], f32, tag="xot")
                            for kt in range(ND):
                                ps = psbig.tile([128, 128], f32, tag="big")
                                nc.tensor.transpose(
                                    ps[:],
                                    sum2T[:, kt * S + st * 128:kt * S + (st + 1) * 128],
                                    ident[:])
                                nc.vector.tensor_tensor(
                                    xo_st[:, kt * 128:(kt + 1) * 128],
                                    ps[:],
                                    trend_tok[:, st * D + kt * 128:st * D + (kt + 1) * 128],
                                    Alu.add)
                            nc.vector.tensor_reduce(
                                stat[:, st:st + 1], xo_st[:],
                                mybir.AxisListType.X, Alu.add)
                            nc.scalar.activation(
                                dump[:], xo_st[:],
                                Act.Square, bias=0.0, scale=1.0,
                                accum_out=statq[:, st:st + 1])
                            hs = slice(st, st + 1)
                            nc.vector.tensor_scalar(mean3[:, hs], stat[:, hs],
                                                    1.0 / D, None, Alu.mult)
                            nc.vector.tensor_scalar(statq[:, hs], statq[:, hs],
                                                    1.0 / D, EPS, Alu.mult,
                                                    Alu.add)
                            nc.vector.tensor_tensor(rstd3[:, hs], mean3[:, hs],
                                                    mean3[:, hs], Alu.mult)
                            nc.vector.tensor_tensor(statq[:, hs], statq[:, hs],
                                                    rstd3[:, hs], Alu.subtract)
                            nc.scalar.activation(rstd3[:, hs], statq[:, hs],
                                                 Act.Sqrt, bias=0.0, scale=1.0)
                            nc.vector.reciprocal(rstd3[:, hs], rstd3[:, hs])
                            nc.vector.tensor_tensor(nb3[:, hs], mean3[:, hs],
                                                    rstd3[:, hs], Alu.mult)
                            nc.vector.tensor_scalar(nb3[:, hs], nb3[:, hs],
                                                    -1.0, None, Alu.mult)
                            xn = scr3.tile([128, D], f32, tag="xn")
                            nc.scalar.activation(
                                xn[:], xo_st[:],
                                Act.Identity, bias=nb3[:, st:st + 1],
                                scale=rstd3[:, st:st + 1])
                            nc.vector.tensor_tensor(xn[:], xn[:], g3bc[:],
                                                    Alu.mult)
                            nc.vector.tensor_tensor(xn[:], xn[:], be3bc[:],
                                                    Alu.add)
                            nc.sync.dma_start(out_d[st * 128:(st + 1) * 128, :],
                                              xn[:])

            wop_p.__exit__(None, None, None)

    nc.compile()
    return nc


def _get_nc():
    if "nc" not in _CACHE:
        _CACHE["nc"] = _build()
    return _CACHE["nc"]


def kernel(**inputs):
    from concourse.bass_utils import run_bass_kernel_spmd

    nc = _get_nc()
    names = ["wq", "bq", "wk", "bk", "wv", "bv", "wo", "bo", "w1", "b1",
             "w2", "b2", "g1", "be1", "g2", "be2", "g3", "be3"]
    shared = {k: np.ascontiguousarray(np.asarray(inputs[k], np.float32))
              for k in names}
    x = np.ascontiguousarray(np.asarray(inputs["x"], np.float32))
    in_maps = [dict(shared, x=x[b]) for b in range(NCORES)]
    res = run_bass_kernel_spmd(nc, in_maps, list(range(NCORES)))
    out = np.stack([res.results[b]["out"] for b in range(NCORES)], axis=0)
    return out.astype(np.float32)


# revision 53
# speedup vs baseline: 1.0342x; 1.0342x over previous
"""Autoformer encoder block on 8 TRN2 NeuronCores.

Sharding: data-parallel over batch (B=8 -> 1 batch per core), weights
replicated. No collectives.

Per-core math (S=1024, D=512, H=8, dp=64, K=25):
  trend = movavg(x)               # banded matmul, token-major
  seas  = x - trend               # token-major, then PE-transpose -> seas.T
  q.T/k.T = wq/wk.T @ seas.T      # feature-major
  v     = seas @ wv               # token-major (for AV lhsT + V-sums)
  The reference's rfft/irfft over the depth axis (n=2S) makes
  corr[b,h,s,t] == 0 for t >= dp, so attention reduces to 64 depth-lags:
    corr.T = IDFT @ (QF (*) conj(KF)), QF = FWD.T @ q.T   (n=128 DFT)
    E = exp(corr/8); out = (E @ v[:64] + (Vsum - Vhead)) / (rowsum(E)+S-dp)
  wo, LN1, FFN(4x, relu), LN2 feature-major (stats via ones-matmul).
  seasonal_out + trend == x_out exactly (trend2 cancels), so movavg2 is
  skipped; final LN3 runs token-major after a PE-transpose, then DMA out.
"""

import numpy as np

B, S, D, H = 8, 1024, 512, 8
DP = D // H
DH = 4 * D
KWIN, PAD = 25, 12
EPS = 1e-6
NCORES = 8
NT = S // 128   # 8 token tiles
ND = D // 128   # 4 feature tiles
NH = DH // 128  # 16 hidden tiles

_CACHE = {}


def _consts():
    c = {}
    # moving-average band blocks: trend[s,:] = sum_t A[t,s] x[t,:]
    # piece 1: t = 128j-12+i, i in [0,128)  (rhs = x_m12 block j)
    # piece 2: t = 128j+116+i, i in [0,24)  (rhs = x_m12 block j+1, rows 0:24)
    cnt = np.minimum(S, np.arange(S) + PAD + 1) - np.maximum(0, np.arange(S) - PAD)
    BAND1 = np.zeros((128, NT * 128), np.float32)
    BAND2 = np.zeros((2 * PAD, NT * 128), np.float32)
    for j in range(NT):
        for cc in range(128):
            s = 128 * j + cc
            for i in range(128):
                t = 128 * j - PAD + i
                if 0 <= t < S and abs(t - s) <= PAD:
                    BAND1[i, 128 * j + cc] = 1.0 / cnt[s]
            for i in range(2 * PAD):
                t = 128 * j + 116 + i
                if 0 <= t < S and abs(t - s) <= PAD:
                    BAND2[i, 128 * j + cc] = 1.0 / cnt[s]
    c["BAND1"], c["BAND2"] = BAND1, BAND2

    # split forward DFT (n=128), doubled over partitions for base-0/64 heads:
    # FWDC [128, 65]: cos(2pi f dd/128), f=0..64; FWDS [128, 64]: sin, f=0..63
    n = 2 * DP
    dd = np.arange(DP)[:, None]
    FWDC = np.cos(2 * np.pi * np.arange(65)[None, :] * dd / n).astype(np.float32)
    FWDS = np.sin(2 * np.pi * np.arange(65)[None, :] * dd / n).astype(np.float32)
    c["FWDC"] = np.concatenate([FWDC, FWDC], axis=0)
    c["FWDS"] = np.concatenate([FWDS, FWDS], axis=0)

    # inverse: corr[t] = IDFT_RE.T @ P_re(f=0..64) + IDFT_IM.T @ P_im(f=0..63)
    t = np.arange(DP)[None, :]
    w = np.full(65, 2.0); w[0] = 1.0; w[64] = 1.0
    fr = np.arange(65)[:, None]
    IRE = (w[:, None] / n) * np.cos(2 * np.pi * fr * t / n)
    fi = np.arange(65)[:, None]
    IIM = -(2.0 / n) * np.sin(2 * np.pi * fi * t / n)
    c["IRE"] = IRE.astype(np.float32)
    c["IIM"] = IIM.astype(np.float32)

    # head-pair selector for 1/Z broadcast: pass p covers heads 2p, 2p+1
    HSEL = np.zeros((8, 4 * 128), np.float32)
    for p in range(4):
        for mm_ in range(128):
            HSEL[2 * p + mm_ // 64, p * 128 + mm_] = 1.0
    c["HSEL"] = HSEL
    # Z-collect one-hot: EH8[:, 8h+m] = (m == h)
    EH8 = np.zeros((128, 64), np.float32)
    for h in range(8):
        EH8[:, 9 * h] = 1.0
    c["EH8"] = EH8

    c["ones"] = np.ones((128, 128), np.float32)
    c["ident"] = np.eye(128, dtype=np.float32)
    return c


def _build():
    import concourse.bacc as bacc
    import concourse.mybir as mybir
    import concourse.tile as tile

    f32 = mybir.dt.float32
    f32r = mybir.dt.float32r
    Alu = mybir.AluOpType
    Act = mybir.ActivationFunctionType

    nc = bacc.Bacc()

    # ---- DRAM parameters (same names as setup_inputs keys) ----
    x_d = nc.declare_dram_parameter("x", [S, D], f32, isOutput=False)
    wq_d = nc.declare_dram_parameter("wq", [D, D], f32, isOutput=False)
    bq_d = nc.declare_dram_parameter("bq", [D], f32, isOutput=False)
    wk_d = nc.declare_dram_parameter("wk", [D, D], f32, isOutput=False)
    bk_d = nc.declare_dram_parameter("bk", [D], f32, isOutput=False)
    wv_d = nc.declare_dram_parameter("wv", [D, D], f32, isOutput=False)
    bv_d = nc.declare_dram_parameter("bv", [D], f32, isOutput=False)
    wo_d = nc.declare_dram_parameter("wo", [D, D], f32, isOutput=False)
    bo_d = nc.declare_dram_parameter("bo", [D], f32, isOutput=False)
    w1_d = nc.declare_dram_parameter("w1", [D, DH], f32, isOutput=False)
    b1_d = nc.declare_dram_parameter("b1", [DH], f32, isOutput=False)
    w2_d = nc.declare_dram_parameter("w2", [DH, D], f32, isOutput=False)
    b2_d = nc.declare_dram_parameter("b2", [D], f32, isOutput=False)
    g1_d = nc.declare_dram_parameter("g1", [D], f32, isOutput=False)
    be1_d = nc.declare_dram_parameter("be1", [D], f32, isOutput=False)
    g2_d = nc.declare_dram_parameter("g2", [D], f32, isOutput=False)
    be2_d = nc.declare_dram_parameter("be2", [D], f32, isOutput=False)
    g3_d = nc.declare_dram_parameter("g3", [D], f32, isOutput=False)
    be3_d = nc.declare_dram_parameter("be3", [D], f32, isOutput=False)
    out_d = nc.declare_dram_parameter("out", [S, D], f32, isOutput=True)

    C = _consts()
    B1_d = nc.inline_tensor(C["BAND1"], name="c_band1")
    B2_d = nc.inline_tensor(C["BAND2"], name="c_band2")
    FWDC_d = nc.inline_tensor(C["FWDC"], name="c_fwdc")
    FWDS_d = nc.inline_tensor(C["FWDS"], name="c_fwds")
    IRE_d = nc.inline_tensor(C["IRE"], name="c_ire")
    IIM_d = nc.inline_tensor(C["IIM"], name="c_iim")
    HSEL_d = nc.inline_tensor(C["HSEL"], name="c_hsel")
    EH8_d = nc.inline_tensor(C["EH8"], name="c_eh8")
    ones_d = nc.inline_tensor(C["ones"], name="c_ones")
    ident_d = nc.inline_tensor(C["ident"], name="c_ident")

    def r(ap):
        return ap.bitcast(f32r)

    def mm(out, lhsT, rhs, start=True, stop=True):
        nc.tensor.matmul(out, r(lhsT), r(rhs), start=start, stop=stop)

    with tile.TileContext(nc) as tc:
        with (
            tc.tile_pool(name="konst", bufs=1) as konst,
            tc.tile_pool(name="persist", bufs=1) as persist,
            tc.tile_pool(name="mid", bufs=1) as mid,
        ):
            # ---- consts to SBUF ----
            fwdc = konst.tile([128, 65], f32)
            fwds = konst.tile([128, 65], f32)
            ire = konst.tile([65, DP], f32)
            iim = konst.tile([65, DP], f32)
            hsel = konst.tile([8, 4 * 128], f32)
            eh8 = konst.tile([128, 64], f32)
            ones = konst.tile([128, 128], f32)
            ident = konst.tile([128, 128], f32)
            for tl, dr in ((fwdc, FWDC_d), (fwds, FWDS_d), (ire, IRE_d),
                           (iim, IIM_d), (hsel, HSEL_d), (eh8, EH8_d),
                           (ones, ones_d)):
                nc.sync.dma_start(r(tl[:]), r(dr[:]))
            nc.sync.dma_start(ident[:], ident_d[:])

            def col(dram, n):
                t = konst.tile([128, n], f32, tag="col_" + dram.name)
                nc.sync.dma_start(t[:], dram.rearrange("(j p) -> p j", p=128))
                return t
            bq_c, bk_c, bv_c = col(bq_d, ND), col(bk_d, ND), col(bv_d, ND)
            bo_c, b2_c = col(bo_d, ND), col(b2_d, ND)
            b1_c = col(b1_d, NH)
            g1_c, be1_c = col(g1_d, ND), col(be1_d, ND)
            g2_c, be2_c = col(g2_d, ND), col(be2_d, ND)
            g3_r = konst.tile([1, D], f32)
            be3_r = konst.tile([1, D], f32)
            nc.sync.dma_start(r(g3_r[:]), r(g3_d[None, :]))
            nc.sync.dma_start(r(be3_r[:]), r(be3_d[None, :]))

            trend_tok = persist.tile([128, NT * D], f32)
            seasT = persist.tile([128, ND * S], f32)
            wqkvo = []

            def wload(pool, dram, din, dout):
                t = pool.tile([128, (din // 128) * dout], f32,
                              tag="w_" + dram.name)
                nc.sync.dma_start(
                    r(t[:].rearrange("p (k f) -> p k f", f=dout)),
                    r(dram.rearrange("(k p) f -> p k f", p=128)))
                return t

            # ============ phase 1: load x, moving average, transpose ========
            wop_p = tc.tile_pool(name="wop", bufs=1)
            wop = wop_p.__enter__()
            wat_ctx = tc.tile_pool(name="wat", bufs=1)
            wat = wat_ctx.__enter__()
            with (
                tc.tile_pool(name="ph1", bufs=1) as ph1,
                tc.tile_pool(name="ps1", bufs=2, space="PSUM") as ps1,
            ):
                band1 = ph1.tile([128, NT * 128], f32)
                band2 = ph1.tile([2 * PAD, NT * 128], f32)
                nc.sync.dma_start(r(band1[:]), r(B1_d[:]))
                nc.sync.dma_start(r(band2[:]), r(B2_d[:]))
                x_m12 = ph1.tile([128, (NT + 1) * D], f32)
                nc.sync.dma_start(r(x_m12[0:PAD, 0:D]), r(x_d[0:PAD, :]))
                nc.sync.dma_start(r(x_m12[PAD:128, 0:D]), r(x_d[0:116, :]))
                nc.sync.dma_start(
                    r(x_m12[:, D:4 * D].rearrange("p (st d) -> p st d", d=D)),
                    r(x_d[116:116 + 3 * 128, :].rearrange(
                        "(st p) d -> p st d", p=128)))
                nc.sync.dma_start(
                    r(x_m12[:, 4 * D:NT * D].rearrange("p (st d) -> p st d", d=D)),
                    r(x_d[116 + 3 * 128:116 + 7 * 128, :].rearrange(
                        "(st p) d -> p st d", p=128)))
                nc.sync.dma_start(r(x_m12[0:PAD, NT * D:(NT + 1) * D]),
                                  r(x_d[S - PAD:S, :]))
                nc.sync.dma_start(r(x_m12[PAD:128, NT * D:(NT + 1) * D]),
                                  r(x_d[0:116, :]))
                x_tok = ph1.tile([128, NT * D], f32)
                for st_ in range(NT):
                    nc.sync.dma_start(
                        x_tok[:, st_ * D:(st_ + 1) * D],
                        x_d[st_ * 128:(st_ + 1) * 128, :])
                for dr_ in (wq_d, wk_d, wv_d):
                    wqkvo.append(wload(wat, dr_, D, D))
                wqkvo.append(wload(wop, wo_d, D, D))
                seas_tok = ph1.tile([128, NT * D], f32)
                for j in range(NT):
                    ps = ps1.tile([128, D], f32, tag="mavg")
                    mm(ps[:], band1[:, 128 * j:128 * (j + 1)],
                       x_m12[:, j * D:(j + 1) * D], start=True, stop=False)
                    mm(ps[:], band2[:, 128 * j:128 * (j + 1)],
                       x_m12[0:2 * PAD, (j + 1) * D:(j + 2) * D],
                       start=False, stop=True)
                    nc.scalar.copy(trend_tok[:, j * D:(j + 1) * D], ps[:])
                    nc.vector.tensor_tensor(
                        seas_tok[:, j * D:(j + 1) * D],
                        x_tok[:, j * D:(j + 1) * D], ps[:], Alu.subtract)

                for st in range(NT):
                    for kt in range(ND):
                        ps = ps1.tile([128, 128], f32, tag="tr")
                        nc.tensor.transpose(
                            ps[:],
                            seas_tok[:, st * D + kt * 128:st * D + (kt + 1) * 128],
                            ident[:])
                        dst = r(seasT[:, kt * S + st * 128:kt * S + (st + 1) * 128])
                        if (st + kt) % 2 == 0:
                            nc.vector.tensor_copy(dst, ps[:])
                        else:
                            nc.scalar.copy(dst, ps[:])

            # ============ phase 2: attention ============
            out1T = mid.tile([128, ND * S], f32, tag="m16")
            with (
                tc.tile_pool(name="ph2", bufs=1) as ph2,
                tc.tile_pool(name="att1", bufs=1) as att1,
                tc.tile_pool(name="scr2", bufs=2) as scr2,
                tc.tile_pool(name="scr2a", bufs=1) as scr2a,
            ):
                wq_s, wk_s, wv_s, wo_s = wqkvo
                bf = mybir.dt.bfloat16
                qT = ph2.tile([128, ND * S], bf)
                kT = ph2.tile([128, ND * S], bf)
                v_tok = ph2.tile([128, NT * D], f32)
                attnT = mid.tile([128, ND * S], f32, tag="attn")

                with tc.tile_pool(name="psqkv", bufs=2, space="PSUM") as psq:
                    for w_s, bcol, dst in ((wq_s, bq_c, qT), (wk_s, bk_c, kT)):
                        for mt in range(ND):
                            ps = psq.tile([128, S], f32, tag="big")
                            for nn in range(2):
                                for k in range(ND):
                                    mm(ps[:, nn * 512:(nn + 1) * 512],
                                       w_s[:, k * D + mt * 128:k * D + (mt + 1) * 128],
                                       seasT[:, k * S + nn * 512:k * S + (nn + 1) * 512],
                                       start=(k == 0), stop=(k == ND - 1))
                            nc.scalar.activation(
                                dst[:, mt * S:(mt + 1) * S], ps[:],
                                Act.Identity, bias=bcol[:, mt:mt + 1], scale=1.0)
                    for st in range(NT):
                        ps = psq.tile([128, D], f32, tag="vtok")
                        for k in range(ND):
                            mm(ps[:], seasT[:, k * S + st * 128:k * S + (st + 1) * 128],
                               wv_s[:, k * D:(k + 1) * D],
                               start=(k == 0), stop=(k == ND - 1))
                        nc.scalar.copy(r(v_tok[:, st * D:(st + 1) * D]), ps[:])

                ire_b = att1.tile([65, DP], bf)
                iim_b = att1.tile([65, DP], bf)
                fwdc_b = att1.tile([128, 65], bf)
                fwds_b = att1.tile([128, 65], bf)
                nc.vector.tensor_copy(ire_b[:], ire[:])
                nc.vector.tensor_copy(iim_b[:], iim[:])
                nc.vector.tensor_copy(fwdc_b[:], fwdc[:])
                nc.vector.tensor_copy(fwds_b[:], fwds[:])
                E_all = att1.tile([DP, H * S], bf)
                scale = float(1.0 / np.sqrt(DP))

                # phase A: DFT -> complex product (bf16) -> corr -> E
                with tc.tile_pool(name="psA2", bufs=2, space="PSUM") as psA2:
                    for h in range(H):
                        kt, ro = h // 2, 64 * (h % 2)
                        odd = (h % 2 == 1)
                        fc = fwdc_b[0:DP, :] if not odd else fwdc_b[DP:128, :]
                        fs = fwds_b[0:DP, :] if not odd else fwds_b[DP:128, :]
                        c0 = kt * S
                        qh = qT[ro:ro + DP, c0:c0 + S]
                        kh = kT[ro:ro + DP, c0:c0 + S]
                        qc = psA2.tile([65, S], f32, tag="fq")
                        qsn = psA2.tile([65, S], f32, tag="fq")
                        kc = psA2.tile([65, S], f32, tag="fk")
                        ksn = psA2.tile([65, S], f32, tag="fk")
                        for nn in range(2):
                            sl = slice(nn * 512, (nn + 1) * 512)
                            nc.tensor.matmul(qc[:, sl], fc, qh[:, sl],
                                             start=True, stop=True)
                            nc.tensor.matmul(qsn[:, sl], fs, qh[:, sl],
                                             start=True, stop=True)
                            nc.tensor.matmul(kc[:, sl], fc, kh[:, sl],
                                             start=True, stop=True)
                            nc.tensor.matmul(ksn[:, sl], fs, kh[:, sl],
                                             start=True, stop=True)
                        qcb = scr2.tile([65, S], bf, tag="qcb")
                        qsb = scr2.tile([65, S], bf, tag="qsb")
                        kcb = scr2.tile([65, S], bf, tag="kcb")
                        ksb = scr2a.tile([65, S], bf, tag="ksb")
                        nc.vector.tensor_copy(qcb[:], qc[:])
                        nc.scalar.copy(qsb[:], qsn[:])
                        nc.vector.tensor_copy(kcb[:], kc[:])
                        nc.scalar.copy(ksb[:], ksn[:])
                        pre = scr2a.tile([65, S], bf, tag="pre")
                        pim = scr2a.tile([65, S], bf, tag="pim")
                        t2 = scr2a.tile([65, S], bf, tag="t2")
                        nc.vector.tensor_tensor(pre[:], qcb[:], kcb[:],
                                                Alu.mult)
                        nc.vector.tensor_tensor(t2[:], qsb[:], ksb[:],
                                                Alu.mult)
                        nc.vector.tensor_tensor(pre[:], pre[:], t2[:], Alu.add)
                        nc.vector.tensor_tensor(pim[:], qcb[:], ksb[:],
                                                Alu.mult)
                        nc.vector.tensor_tensor(t2[:], qsb[:], kcb[:],
                                                Alu.mult)
                        nc.vector.tensor_tensor(pim[:], pim[:], t2[:],
                                                Alu.subtract)
                        cr = psA2.tile([DP, S], f32, tag="fk")
                        for nn in range(2):
                            sl = slice(nn * 512, (nn + 1) * 512)
                            nc.tensor.matmul(cr[:, sl], ire_b[:], pre[:, sl],
                                             start=True, stop=False)
                            nc.tensor.matmul(cr[:, sl], iim_b[:], pim[:, sl],
                                             start=False, stop=True)
                        nc.scalar.activation(E_all[:, h * S:(h + 1) * S],
                                             cr[:], Act.Exp, bias=0.0,
                                             scale=scale)

                # W = Vsum - Vhead = (sum_{s>=64} seas.T[:,s]) @ wv
                W_sb = att1.tile([128, ND], f32)
                sdif = att1.tile([128, 2 * ND], f32)
                dmp = scr2a.tile([128, S - DP], f32, tag="dmp")
                for k in range(ND):
                    with nc.allow_low_precision(reason="f32r rhs"):
                        nc.scalar.activation(
                            dmp[:], seasT[:, k * S + DP:(k + 1) * S],
                            Act.Copy,
                            accum_out=r(sdif[:, 2 * k:2 * k + 1]))
                    nc.vector.tensor_copy(r(sdif[:, 2 * k + 1:2 * k + 2]),
                                          sdif[:, 2 * k:2 * k + 1])
                with tc.tile_pool(name="psw", bufs=1, space="PSUM") as psw:
                    ps_w2 = psw.tile([128, 2 * ND], f32, tag="w2")
                    for mt in range(ND):
                        for k in range(ND):
                            mm(ps_w2[:, 2 * mt:2 * mt + 2],
                               wv_s[:, k * D + mt * 128:k * D + (mt + 1) * 128],
                               sdif[:, 2 * k:2 * k + 2],
                               start=(k == 0), stop=(k == ND - 1))
                    nc.vector.tensor_copy(W_sb[:],
                                          ps_w2[:].rearrange("p (a b) -> p a b", b=2)[:, :, 0])

                # phase B: AV + Z-collect -> batched 1/Z -> scale + bv
                vb = att1.tile([DP, H * DP + 1], bf)
                nc.vector.tensor_copy(vb[:, 0:H * DP], v_tok[0:DP, 0:H * DP])
                nc.vector.tensor_copy(vb[:, H * DP:H * DP + 1], ones[0:DP, 0:1])
                zrs = att1.tile([128, S], f32)
                with (
                    tc.tile_pool(name="psB2", bufs=2, space="PSUM") as psB2,
                    tc.tile_pool(name="psZ", bufs=1, space="PSUM") as psZ,
                ):
                    Zall = psZ.tile([8, S], f32, tag="zall")
                    for h in range(H):
                        kt, ro = h // 2, 64 * (h % 2)
                        odd = (h % 2 == 1)
                        c0 = kt * S
                        if not odd:
                            avl = scr2.tile([DP, 65], bf, tag="avle")
                            nc.vector.tensor_copy(
                                avl[:, 0:DP], vb[:, DP * h:DP * h + DP])
                            nc.vector.tensor_copy(
                                avl[:, DP:65], vb[:, H * DP:H * DP + 1])
                        else:
                            avl = scr2.tile([DP, 128], bf, tag="avlo")
                            nc.vector.tensor_copy(
                                avl[:, 0:1], vb[:, H * DP:H * DP + 1])
                            nc.vector.tensor_scalar(avl[:, 1:64],
                                                    vb[:, 1:64], 0.0,
                                                    None, Alu.mult)
                            nc.vector.tensor_copy(
                                avl[:, 64:128], vb[:, DP * h:DP * h + DP])
                        nv = psB2.tile([128, S], f32, tag="nv")
                        for nn in range(2):
                            sl = slice(nn * 512, (nn + 1) * 512)
                            nc.tensor.matmul(
                                nv[0:avl.shape[1], sl], avl[:],
                                E_all[:, h * S + nn * 512:h * S + (nn + 1) * 512],
                                start=True, stop=True)
                        zrow = 64 if not odd else 0
                        nrows = slice(0, DP) if not odd else slice(64, 128)
                        nc.scalar.activation(r(zrs[zrow:zrow + 1, :]),
                                             nv[zrow:zrow + 1, :], Act.Copy)
                        for nn in range(2):
                            sl = slice(nn * 512, (nn + 1) * 512)
                            mm(Zall[:, sl], eh8[zrow:zrow + 1, 8 * h:8 * h + 8],
                               zrs[zrow:zrow + 1, sl],
                               start=(h == 0), stop=(h == H - 1))
                        nc.vector.tensor_scalar(
                            r(attnT[ro:ro + DP, c0:c0 + S]), nv[nrows, :],
                            W_sb[ro:ro + DP, kt:kt + 1], None, Alu.add)
                    zsum = att1.tile([8, S], f32)
                    zinv = att1.tile([8, S], f32)
                    zinv_b = att1.tile([8, S], bf)
                    hsel_b = att1.tile([8, 4 * 128], bf)
                    nc.vector.tensor_copy(hsel_b[:], hsel[:])
                    nc.vector.tensor_scalar(zsum[:], Zall[:], float(S - DP),
                                            None, Alu.add)
                    nc.scalar.activation(zinv[:], zsum[:], Act.Ln, bias=0.0,
                                         scale=1.0)
                    nc.scalar.activation(zinv_b[:], zinv[:], Act.Exp, bias=0.0,
                                         scale=-1.0)
                with tc.tile_pool(name="psZb", bufs=2, space="PSUM") as psZb:
                    for p in range(4):
                        zbc = psZb.tile([128, S], f32, tag="zbc")
                        for nn in range(2):
                            sl = slice(nn * 512, (nn + 1) * 512)
                            nc.tensor.matmul(zbc[:, sl],
                                             hsel_b[:, p * 128:(p + 1) * 128],
                                             zinv_b[:, sl],
                                             start=True, stop=True)
                        nc.vector.tensor_tensor(r(attnT[:, p * S:(p + 1) * S]),
                                                attnT[:, p * S:(p + 1) * S],
                                                zbc[:], Alu.mult)
                        nc.vector.tensor_scalar(r(attnT[:, p * S:(p + 1) * S]),
                                                attnT[:, p * S:(p + 1) * S],
                                                bv_c[:, p:p + 1], None, Alu.add)

            wat_ctx.__exit__(None, None, None)
            out1T_ = out1T

            # ============ layernorm helper (feature-major, per token-half) ==
            def layernorm_T(psln, scr, scr1, src, dst, g_c, be_c, h0):
                if True:
                    s1 = psln.tile([1, 512], f32, tag="stat1")
                    s2 = psln.tile([1, 512], f32, tag="stat2")
                    for k in range(ND):
                        c0 = k * S + h0
                        sq = scr.tile([128, 512], f32, tag="lnsq")
                        nc.vector.tensor_tensor(r(sq[:]), src[:, c0:c0 + 512],
                                                src[:, c0:c0 + 512], Alu.mult)
                        mm(s1[:], ones[:, 0:1], src[:, c0:c0 + 512],
                           start=(k == 0), stop=(k == ND - 1))
                        mm(s2[:], ones[:, 0:1], sq[:],
                           start=(k == 0), stop=(k == ND - 1))
                    mean = scr1.tile([1, 512], f32, tag="lnm")
                    msq = scr1.tile([1, 512], f32, tag="lnq")
                    var = scr1.tile([1, 512], f32, tag="lnv")
                    sd = scr1.tile([1, 512], f32, tag="lnq")
                    rstd = scr1.tile([1, 512], f32, tag="lnr")
                    bbn = scr1.tile([1, 512], f32, tag="lnv")
                    bb = scr1.tile([1, 512], f32, tag="lnm")
                    nc.vector.tensor_scalar(mean[:], s1[:], 1.0 / D, None,
                                            Alu.mult)
                    nc.vector.tensor_scalar(var[:], s2[:], 1.0 / D, EPS,
                                            Alu.mult, Alu.add)
                    nc.vector.tensor_tensor(msq[:], mean[:], mean[:], Alu.mult)
                    nc.vector.tensor_tensor(var[:], var[:], msq[:],
                                            Alu.subtract)
                    nc.scalar.activation(sd[:], var[:], Act.Ln, bias=0.0,
                                         scale=1.0)
                    nc.scalar.activation(r(rstd[:]), sd[:], Act.Exp, bias=0.0,
                                         scale=-0.5)
                    nc.vector.tensor_scalar(bbn[:], mean[:], -1.0, None,
                                            Alu.mult)
                    nc.vector.tensor_tensor(r(bb[:]), bbn[:], rstd[:], Alu.mult)
                    abc = psln.tile([128, 512], f32, tag="lnA")
                    bbc = psln.tile([128, 512], f32, tag="lnB")
                    mm(abc[:], ones[0:1, :], rstd[:])
                    mm(bbc[:], ones[0:1, :], bb[:])
                    for k in range(ND):
                        c0 = k * S + h0
                        t = scr.tile([128, 512], f32, tag="lnt")
                        nc.vector.tensor_tensor(t[:], src[:, c0:c0 + 512],
                                                abc[:], Alu.mult)
                        nc.vector.tensor_tensor(t[:], t[:], bbc[:], Alu.add)
                        nc.scalar.activation(r(dst[:, c0:c0 + 512]), t[:],
                                             Act.Identity,
                                             bias=be_c[:, k:k + 1],
                                             scale=g_c[:, k:k + 1])

            # ============ back end: wo -> LN1 -> FFN -> LN2 -> LN3, 2 halves =
            with (
                tc.tile_pool(name="ph3", bufs=2) as ph3,
                tc.tile_pool(name="scr3", bufs=2) as scr3,
                tc.tile_pool(name="scr3s", bufs=1) as scr3s,
                tc.tile_pool(name="w1p", bufs=1) as w1p,
                tc.tile_pool(name="w2p", bufs=1) as w2p,
                tc.tile_pool(name="hTp", bufs=1) as hTp,
                tc.tile_pool(name="psbig", bufs=4, space="PSUM") as psbig,
                tc.tile_pool(name="psst", bufs=1, space="PSUM") as psst,
            ):
                w1_s = wload(w1p, w1_d, D, DH)
                w2_s = wload(w2p, w2_d, DH, D)
                sum2T = attnT
                g3p = psbig.tile([128, D], f32, tag="big")
                be3p = psbig.tile([128, D], f32, tag="big")
                mm(g3p[:], ones[0:1, :], g3_r[:])
                mm(be3p[:], ones[0:1, :], be3_r[:])
                g3bc = scr3s.tile([128, D], f32, tag="g3bc")
                be3bc = scr3s.tile([128, D], f32, tag="be3bc")
                nc.vector.tensor_copy(g3bc[:], g3p[:])
                nc.vector.tensor_copy(be3bc[:], be3p[:])
                stat = scr3s.tile([128, NT], f32, tag="st3s")
                statq = scr3s.tile([128, NT], f32, tag="st3q")
                mean3 = scr3s.tile([128, NT], f32, tag="st3m")
                rstd3 = scr3s.tile([128, NT], f32, tag="st3r")
                nb3 = scr3s.tile([128, NT], f32, tag="st3nb")
                dump = scr3.tile([128, D], f32, tag="xn")

                for hf in range(2):
                    h0 = hf * 512
                    # wo + residual
                    if True:
                        for mt in range(ND):
                            ps = psbig.tile([128, 512], f32, tag="big")
                            for k in range(ND):
                                mm(ps[:],
                                   wo_s[:, k * D + mt * 128:k * D + (mt + 1) * 128],
                                   attnT[:, k * S + h0:k * S + h0 + 512],
                                   start=(k == 0), stop=(k == ND - 1))
                            tmp = scr3s.tile([128, 512], f32, tag="evac")
                            nc.scalar.activation(tmp[:], ps[:], Act.Identity,
                                                 bias=bo_c[:, mt:mt + 1],
                                                 scale=1.0)
                            nc.vector.tensor_tensor(
                                r(out1T_[:, mt * S + h0:mt * S + h0 + 512]),
                                tmp[:], seasT[:, mt * S + h0:mt * S + h0 + 512],
                                Alu.add)
                    layernorm_T(psst, scr3, scr3s, out1T_, out1T_, g1_c, be1_c, h0)
                    # FFN
                    if True:
                        hTa = hTp.tile([128, 8 * 512], f32, tag="hTa")
                        hTb = hTp.tile([128, 8 * 512], f32, tag="hTb")
                        hTs = [hTa, hTb]
                        for mt in range(NH):
                            ps = psbig.tile([128, 512], f32, tag="big")
                            for k in range(ND):
                                mm(ps[:],
                                   w1_s[:, k * DH + mt * 128:k * DH + (mt + 1) * 128],
                                   out1T_[:, k * S + h0:k * S + h0 + 512],
                                   start=(k == 0), stop=(k == ND - 1))
                            ht_dst = hTs[mt // 8][:, (mt % 8) * 512:(mt % 8 + 1) * 512]
                            if mt % 2 == 0:
                                nc.scalar.activation(
                                    r(ht_dst), ps[:],
                                    Act.Relu, bias=b1_c[:, mt:mt + 1], scale=1.0)
                            else:
                                nc.vector.tensor_scalar(
                                    r(ht_dst), ps[:],
                                    b1_c[:, mt:mt + 1], 0.0, Alu.add, Alu.max)
                        for mt in range(ND):
                            ps = psbig.tile([128, 512], f32, tag="big")
                            for k in range(NH):
                                mm(ps[:],
                                   w2_s[:, k * D + mt * 128:k * D + (mt + 1) * 128],
                                   hTs[k // 8][:, (k % 8) * 512:(k % 8 + 1) * 512],
                                   start=(k == 0), stop=(k == NH - 1))
                            tmp = scr3s.tile([128, 512], f32, tag="evac")
                            nc.scalar.activation(tmp[:], ps[:], Act.Identity,
                                                 bias=b2_c[:, mt:mt + 1],
                                                 scale=1.0)
                            nc.vector.tensor_tensor(
                                r(sum2T[:, mt * S + h0:mt * S + h0 + 512]),
                                tmp[:],
                                out1T_[:, mt * S + h0:mt * S + h0 + 512],
                                Alu.add)
                    layernorm_T(psst, scr3, scr3s, sum2T, sum2T, g2_c, be2_c, h0)
                    for k in range(ND):
                        c0 = k * S + h0
                        nc.vector.tensor_tensor(r(sum2T[:, c0:c0 + 512]),
                                                sum2T[:, c0:c0 + 512],
                                                seasT[:, c0:c0 + 512], Alu.add)
                    # transpose back + trend, LN3 token-major, store
                    if True:
                        for st in range(hf * 4, hf * 4 + 4):
                            xo_st = ph3.tile([128, D], f32, tag="xot")
                            for kt in range(ND):
                                ps = psbig.tile([128, 128], f32, tag="big")
                                nc.tensor.transpose(
                                    ps[:],
                                    sum2T[:, kt * S + st * 128:kt * S + (st + 1) * 128],
                                    ident[:])
                                nc.vector.tensor_tensor(
                                    xo_st[:, kt * 128:(kt + 1) * 128],
                                    ps[:],
                                    trend_tok[:, st * D + kt * 128:st * D + (kt + 1) * 128],
                                    Alu.add)
                            nc.vector.tensor_reduce(
                                stat[:, st:st + 1], xo_st[:],
                                mybir.AxisListType.X, Alu.add)
                            nc.scalar.activation(
                                dump[:], xo_st[:],
                                Act.Square, bias=0.0, scale=1.0,
                                accum_out=statq[:, st:st + 1])
                            hs = slice(st, st + 1)
                            nc.vector.tensor_scalar(mean3[:, hs], stat[:, hs],
                                                    1.0 / D, None, Alu.mult)
                            nc.vector.tensor_scalar(statq[:, hs], statq[:, hs],
                                                    1.0 / D, EPS, Alu.mult,
                                                    Alu.add)
                            nc.vector.tensor_tensor(rstd3[:, hs], mean3[:, hs],
                                                    mean3[:, hs], Alu.mult)
                            nc.vector.tensor_tensor(statq[:, hs], statq[:, hs],
                                                    rstd3[:, hs], Alu.subtract)
                            nc.scalar.activation(rstd3[:, hs], statq[:, hs],
                                                 Act.Sqrt, bias=0.0, scale=1.0)
                            nc.vector.reciprocal(rstd3[:, hs], rstd3[:, hs])
                            nc.vector.tensor_tensor(nb3[:, hs], mean3[:, hs],
                                                    rstd3[:, hs], Alu.mult)
                            nc.vector.tensor_scalar(nb3[:, hs], nb3[:, hs],
                                                    -1.0, None, Alu.mult)
                            xn = scr3.tile([128, D], f32, tag="xn")
                            nc.scalar.activation(
                                xn[:], xo_st[:],
                                Act.Identity, bias=nb3[:, st:st + 1],
                                scale=rstd3[:, st:st + 1])
                            nc.vector.tensor_tensor(xn[:], xn[:], g3bc[:],
                                                    Alu.mult)
                            nc.vector.tensor_tensor(xn[:], xn[:], be3bc[:],
                                                    Alu.add)
                            nc.sync.dma_start(out_d[st * 128:(st + 1) * 128, :],
                                              xn[:])

            wop_p.__exit__(None, None, None)

    nc.compile()
    return nc


def _get_nc():
    if "nc" not in _CACHE:
        _CACHE["nc"] = _build()
    return _CACHE["nc"]


def kernel(**inputs):
    from concourse.bass_utils import run_bass_kernel_spmd

    nc = _get_nc()
    names = ["wq", "bq", "wk", "bk", "wv", "bv", "wo", "bo", "w1", "b1",
             "w2", "b2", "g1", "be1", "g2", "be2", "g3", "be3"]
    shared = {k: np.ascontiguousarray(np.asarray(inputs[k], np.float32))
              for k in names}
    x = np.ascontiguousarray(np.asarray(inputs["x"], np.float32))
    in_maps = [dict(shared, x=x[b]) for b in range(NCORES)]
    res = run_bass_kernel_spmd(nc, in_maps, list(range(NCORES)))
    out = np.stack([res.results[b]["out"] for b in range(NCORES)], axis=0)
    return out.astype(np.float32)
